# revision 26
# baseline (speedup 1.0000x reference)
"""TRN2 Bass kernel: fused multi-head attention (GPT-2 style, no causal mask).

Computes, for full inputs:
    qkv = X @ c_attn_w + c_attn_b ; q,k,v = split(qkv)
    per head: P = softmax(q k^T / sqrt(64)) ; a = P v
    out = merge_heads(a) @ c_proj_w + c_proj_b

Sharding: tensor-parallel over heads. 16 heads across 8 cores -> 2 heads/core.
Partials summed on the host (fp16 partials, fp32 host accumulate).

v2: single software-pipelined loop. Per inner step the PE issues, in order:
the previous-kb OT (P@V) pair, three QKV-projection matmuls for the NEXT
batch, the row-tiled S^T pair for both heads (concurrent row groups 0-63 /
64-127), and one c_proj matmul for the PREVIOUS batch. This keeps the PE
back-to-back busy so the HAM clock gate stays at K=8/8 (the baseline spent
~140us at half clock). The two S^T matmuls land in one [128,1024] PSUM tile
so a single ACTIVATE does the exp for both heads (64 ACTIVATEs instead of
128). Softmax denominators ride the OT matmul as a ones-column (row 64);
reciprocal is a 3-op DVE chain (bit-trick seed + 1 Newton step) whose result
row is broadcast to 64 partitions with a stride-0 DMA replicate, avoiding
both a PE broadcast matmul and a PSUM bank.
"""

import os
from contextlib import ExitStack

import ml_dtypes
import numpy as np

import concourse.bass as bass
import concourse.mybir as mybir
from concourse import bacc, tile
from concourse.bass_utils import run_bass_kernel_spmd

F32 = mybir.dt.float32
F32R = mybir.dt.float32r
BF16 = mybir.dt.bfloat16
F16 = mybir.dt.float16
I32 = mybir.dt.int32

B, S, NX = 4, 1024, 1024
T = B * S  # 4096 tokens
NCORES = 8
HD = 64  # head dim
V2S = 2 * (HD + 1)  # 130 columns per 128-token block in the V2 layout
EXP = mybir.ActivationFunctionType.Exp
RECIP_MAGIC = 0x7EF311C3  # fp32 bit-trick reciprocal seed, ~5% rel err
NEG_MAGIC = RECIP_MAGIC - 0x80000000  # same seed with the sign bit set

_nc_cache = None


def _ensure_ntff_hook():
    """The agent image's `antenv` lacks `axon_hooks`; synthesize it (see
    trn_agent_boot). Returns True if profiling is available."""
    import sys
    import types

    try:
        from antenv.axon_hooks import get_axon_ntff_profile_hook  # noqa: F401

        return True
    except ImportError:
        pass
    try:
        import antenv
        from trn_agent_boot.trn_boot import _ntff_profile_via_ctypes

        mod = types.ModuleType("antenv.axon_hooks")
        mod._hook = _ntff_profile_via_ctypes("/opt/axon/libaxon_pjrt.so")

        def set_axon_ntff_profile_hook(h):
            mod._hook = h

        def get_axon_ntff_profile_hook():
            return mod._hook

        mod.set_axon_ntff_profile_hook = set_axon_ntff_profile_hook
        mod.get_axon_ntff_profile_hook = get_axon_ntff_profile_hook
        sys.modules["antenv.axon_hooks"] = mod
        antenv.axon_hooks = mod
        return True
    except Exception as e:  # pragma: no cover - profiling is best-effort
        print(f"kernel.py: NTFF profile hook unavailable ({e}); running untraced")
        return False


def _emit(nc, tc, xtr, wq, wk, wv, wp, bq, bk, bv, identd, onecd, onefd, out):
    with ExitStack() as ctx:
        const = ctx.enter_context(tc.tile_pool(name="const", bufs=1))
        wq_sb = const.tile([128, 1024], BF16, tag="wq")
        wk_sb = const.tile([128, 1024], BF16, tag="wk")
        wv_sb = const.tile([128, 1024], BF16, tag="wv")
        wp_sb = const.tile([128, 1024], BF16, tag="wp")
        bq_sb = const.tile([128, 1], F32, tag="bq")
        bk_sb = const.tile([128, 1], F32, tag="bk")
        bv_sb = const.tile([128, 1], F32, tag="bv")
        ident = const.tile([128, 128], BF16, tag="ident")
        onec = const.tile([128, 64], BF16, tag="onec")
        onef = const.tile([128, 64], BF16, tag="onef")
        # full X^T resident in SBUF: xt_all[p, qc, k, n] = X[qc*512+n, k*128+p]
        # (qc-major so each per-qc DMA is one contiguous 8KB run per partition)
        xt_all = const.tile([128, 8, 8, 512], BF16, tag="xt_all")
        qt = const.tile([128, T], BF16, tag="qt")
        kt = const.tile([128, T], BF16, tag="kt")
        vt = const.tile([128, T], BF16, tag="vt")
        v2 = const.tile([128, (T // 128) * V2S], BF16, tag="v2")
        atall = const.tile([128, T], BF16, tag="atall")
        at1 = const.tile([64, T], BF16, tag="at1")

        # ident + weights first (the PE warmup and the first QKV fills need
        # them), then the xt stream in q-chunk order; each xt DMA is one
        # contiguous 8KB-per-partition run (128 descriptors, cheap issue).
        nc.sync.dma_start(ident[:], identd)
        nc.sync.dma_start(wq_sb[:], wq)
        nc.sync.dma_start(wk_sb[:], wk)
        nc.sync.dma_start(wv_sb[:], wv)
        nc.sync.dma_start(bq_sb[:], bq)
        nc.sync.dma_start(bk_sb[:], bk)
        nc.sync.dma_start(bv_sb[:], bv)
        for qc in range(2):
            nc.sync.dma_start(xt_all[:, qc, :, :], xtr[:, qc, :, :])
        nc.sync.dma_start(wp_sb[:], wp)
        nc.sync.dma_start(onec[:], onecd)
        nc.sync.dma_start(onef[:], onefd)
        for qc in range(2, 8):
            nc.sync.dma_start(xt_all[:, qc, :, :], xtr[:, qc, :, :])
        # ones columns of V2: per token block, col 64 (head 0) and col 129 (head 1)
        v2_ones = v2[:].rearrange("p (t h e) -> p t h e", h=2, e=HD + 1)[
            :, :, :, HD : HD + 1
        ]
        nc.vector.tensor_copy(
            v2_ones, onec[:].rearrange("p (t h e) -> p t h e", h=2, e=1)
        )

        stP = ctx.enter_context(tc.tile_pool(name="stP", bufs=2, space="PSUM"))
        qkvP = ctx.enter_context(tc.tile_pool(name="qkvP", bufs=1, space="PSUM"))
        msP = ctx.enter_context(tc.tile_pool(name="msP", bufs=1, space="PSUM"))
        otP = ctx.enter_context(tc.tile_pool(name="otP", bufs=2, space="PSUM"))
        ptp = ctx.enter_context(tc.tile_pool(name="ptp", bufs=6))
        rcp = ctx.enter_context(tc.tile_pool(name="rcp", bufs=6))
        bcp = ctx.enter_context(tc.tile_pool(name="bcp", bufs=2))
        obp = ctx.enter_context(tc.tile_pool(name="obp", bufs=16))

        # ---- PE warmup: keep the PE busy while xt streams in, so the HAM
        # clock gate reaches K=8/8 before the real matmuls start.
        warm = stP.tile([128, 1024], F32, tag="st", name="warm")
        for _ in range(70):
            nc.tensor.matmul(warm[:, 0:128], ident[:], ident[:], start=True, stop=True)

        # ---------- QKV machinery (for one batch = 48 matmuls, 6 fills) ----
        w_for = {0: wq_sb, 1: wk_sb, 2: wv_sb}
        b_for = {0: bq_sb, 1: bk_sb, 2: bv_sb}
        d_for = {0: qt, 1: kt, 2: vt}
        qkv_state = {}  # live psum tile for the running fill

        def emit_qkv_mm(bn, m):
            """m-th of the 48 QKV matmuls for batch bn; returns pending
            (transpose work) when a v-fill completes."""
            fill, chunk = divmod(m, 8)
            qc = 2 * bn + fill // 3
            tgt = fill % 3
            if chunk == 0:
                qkv_state["ps"] = qkvP.tile([128, 512], F32, tag="fill", name="ps")
            ps = qkv_state["ps"]
            nc.tensor.matmul(
                ps[:],
                w_for[tgt][:, chunk * 128 : (chunk + 1) * 128],
                xt_all[:, qc, chunk, :],
                start=(chunk == 0),
                stop=(chunk == 7),
            )
            if chunk == 7:
                # PSUM->SBUF cast + per-partition bias; alternate engines
                dst = d_for[tgt]
                if fill % 2 == 0:
                    nc.scalar.activation(
                        dst[:, qc * 512 : (qc + 1) * 512],
                        ps[:],
                        mybir.ActivationFunctionType.Identity,
                        bias=b_for[tgt][:, 0:1],
                    )
                else:
                    nc.vector.tensor_scalar_add(
                        dst[:, qc * 512 : (qc + 1) * 512], ps[:], b_for[tgt][:, 0:1]
                    )
                if tgt == 2:
                    return qc  # v-fill complete -> transposes pending
            return None

        def emit_v_transposes(qc):
            """V^T -> token-major V2 for one q-chunk (4 PE transposes + copy)."""
            tp = msP.tile([128, 512], BF16, tag="msc", name="tp")
            for t4 in range(4):
                nc.tensor.transpose(
                    tp[:, t4 * 128 : (t4 + 1) * 128],
                    vt[:, qc * 512 + t4 * 128 : qc * 512 + (t4 + 1) * 128],
                    ident[:],
                )
            src = tp[:].rearrange("p (t h e) -> p t h e", h=2, e=HD)
            dst = v2[:].rearrange("p (t h e) -> p t h e", h=2, e=HD + 1)[
                :, qc * 4 : (qc + 1) * 4, :, 0:HD
            ]
            nc.vector.tensor_copy(dst, src)

        # ---------- prologue: QKV for batch 0, dense on the PE ----------
        pend_tp = []
        for m in range(48):
            r = emit_qkv_mm(0, m)
            if r is not None:
                pend_tp.append(r)
        for qc in pend_tp:
            emit_v_transposes(qc)
        pend_tp = []

        # ---------- softmax tail helpers ----------
        def emit_recip(g, h, den_row):
            """3-op DVE chain on the [1,512] denominator row (row 64 of the
            OT psum tile): bit-trick seed + one Newton step -> m1 = -1/den."""
            sd = rcp.tile([65, 512], I32, tag="sd", name="sd")
            nc.vector.tensor_scalar(
                sd[64:65, :],
                den_row.bitcast(I32),
                -1,
                RECIP_MAGIC,
                mybir.AluOpType.mult,
                mybir.AluOpType.add,
            )
            y0 = sd[64:65, :].bitcast(F32)
            t1 = rcp.tile([65, 512], F32, tag="t1", name="t1")
            nc.vector.tensor_mul(t1[64:65, :], den_row, y0)
            m1 = rcp.tile([65, 512], BF16, tag="m1", name="m1")  # m1 = -y1 ~ -1/den
            nc.vector.scalar_tensor_tensor(
                m1[64:65, :],
                t1[64:65, :],
                2.0,
                y0,
                mybir.AluOpType.subtract,
                mybir.AluOpType.mult,
            )
            return m1

        def emit_bcast(m1):
            """Broadcast -1/den to 64 partitions: PE matmul with a -1 ones
            row (K=1) -> PSUM (+1/den), then ACT copy to SBUF."""
            bcb = msP.tile([64, 512], F32, tag="msc", name="bcb")
            nc.tensor.matmul(
                bcb[:],
                onef[64:65, 0:64],
                m1[64:65, :],
                start=True,
                stop=True,
            )
            bc = bcp.tile([64, 512], F32, tag="bc", name="bc")
            nc.scalar.copy(bc[:], bcb[:])
            return bc

        def emit_norm(g, h, ot_h, bc):
            """atall[h] = ot * (1/den), written bf16."""
            b_, q2_ = divmod(g, 2)
            q0 = b_ * 1024 + q2_ * 512
            at = atall if h == 0 else at1
            nc.vector.tensor_mul(at[0:64, q0 : q0 + 512], ot_h[0:64, :], bc[0:64, :])
            if h == 1:
                nc.scalar.dma_start(
                    atall[64:128, q0 : q0 + 512], at1[0:64, q0 : q0 + 512]
                )

        # ---------- cproj helpers ----------
        # cproj tiles are scheduled from a FIFO with a per-group quota so the
        # final (QKV-less) groups stay dense; each [128,512] tile is copied
        # right after its mm (engines alternating) and DMA'd at group ends.
        cproj_fifo = []  # (cb, oc, t2) ready to compute
        cproj_done = []  # (cb, oc, t2, ob_tile) copied, awaiting DMA
        cproj_eng = [0]

        def emit_cproj(cb, oc, t2):
            op_t = msP.tile([128, 512], F32, tag="msc", name="op")
            nc.tensor.matmul(
                op_t[:],
                wp_sb[:, oc * 128 : (oc + 1) * 128],
                atall[:, cb * 1024 + t2 * 512 : cb * 1024 + (t2 + 1) * 512],
                start=True,
                stop=True,
            )
            ob = obp.tile([128, 512], F16, tag="ob", name="ob")
            cproj_eng[0] ^= 1
            if cproj_eng[0]:
                nc.vector.tensor_copy(ob[:], op_t[:])
            else:
                nc.scalar.copy(ob[:], op_t[:])
            cproj_done.append((cb, oc, t2, ob))

        def emit_out_dmas():
            while cproj_done:
                cb, oc, t2, ob = cproj_done.pop(0)
                nc.sync.dma_start(
                    out[
                        oc * 128 : (oc + 1) * 128,
                        cb * 1024 + t2 * 512 : cb * 1024 + (t2 + 1) * 512,
                    ],
                    ob[:],
                )

        # ---------- main pipelined loop over the 8 attention groups -------
        # group g = (b, q2): 512 query tokens, steps s = kb = 0..7.
        # Per step the PE issues, in order: the lag-4 OT pair, one QKV mm for
        # the NEXT batch, the row-tiled S^T pair (g, s), two more QKV mms,
        # pending V transposes, and one c_proj mm for the PREVIOUS batch.
        # exp(g, s) follows the S^T pair on ACT (ACT runs exps + the few
        # PSUM->SBUF copies that must not crowd them).  At s==7 the remaining
        # OT pairs (kb 4..7) are bunched with filler in between so exp(g,7)
        # has finished when OT(g,7) issues.  The softmax tail of group g
        # (reciprocal chain) is emitted at the end of s==7; its broadcast +
        # normalize land in the first steps of group g+1.
        CPROJ_QUOTA = (0, 0, 1, 1, 2, 2, 1, 1)
        pts = {}  # (g, kb) -> pt tile [128, 1024] (h0 cols 0-511, h1 512-1023)
        ots = {}  # (g, h) -> ot psum tile
        m1s = {}  # (g, h) -> recip row tile

        def v2_col(b_, kb, h):
            return ((b_ * 8 + kb) * 2 + h) * (HD + 1)

        def emit_ot_pair(g, kb):
            b_ = g // 2
            for h in (0, 1):
                if kb == 0:
                    ots[(g, h)] = otP.tile([65, 512], F32, tag="ot", name="ot")
                c = v2_col(b_, kb, h)
                nc.tensor.matmul(
                    ots[(g, h)][:],
                    v2[:, c : c + HD + 1],
                    pts[(g, kb)][:, h * 512 : (h + 1) * 512],
                    start=(kb == 0),
                    stop=(kb == 7),
                )

        def emit_st_pair(g, s):
            b_, q2_ = divmod(g, 2)
            q0 = b_ * 1024 + q2_ * 512
            k0 = b_ * 1024 + s * 128
            st = stP.tile([128, 1024], F32, tag="st", name="st")
            nc.tensor.matmul(
                st[:, 0:512],
                kt[0:64, k0 : k0 + 128],
                qt[0:64, q0 : q0 + 512],
                start=True,
                stop=True,
            )
            nc.tensor.matmul(
                st[:, 512:1024],
                kt[64:128, k0 : k0 + 128],
                qt[64:128, q0 : q0 + 512],
                start=True,
                stop=True,
            )
            pt = ptp.tile([128, 1024], BF16, tag="pt", name="pt")
            nc.scalar.activation(pt[:], st[:], EXP, scale=0.125)
            pts[(g, s)] = pt

        def emit_group_tail_recips(g):
            """End of group g (all OT done): reciprocal chains for both heads."""
            for h in (0, 1):
                m1s[(g, h)] = emit_recip(g, h, ots[(g, h)][64:65, :])

        def emit_group_tail_finish(g, s):
            """During group g+1 steps 0-2: broadcast + normalize of group g."""
            if s <= 1:
                h = s
                bc = emit_bcast(m1s.pop((g, h)))
                emit_norm(g, h, ots[(g, h)], bc)
                if h == 1:
                    ots.pop((g, 0))
                    ots.pop((g, 1))

        for g in range(8):
            b_, q2_ = divmod(g, 2)
            nb = b_ + 1  # batch whose QKV we compute this group
            cb = b_ - 1  # batch whose cproj we compute this group

            for s in range(8):
                # --- OT pair, lag 4 ---
                if s >= 4:
                    emit_ot_pair(g, s - 4)
                # --- QKV filler for next batch ---
                if nb <= 3:
                    base = (q2_ * 8 + s) * 3
                    r = emit_qkv_mm(nb, base)
                    if r is not None:
                        pend_tp.append(r)
                # --- S^T pair + exp ---
                emit_st_pair(g, s)
                # --- more QKV filler after the S^T so the st WAR never stalls
                if nb <= 3:
                    for j in (1, 2):
                        r = emit_qkv_mm(nb, base + j)
                        if r is not None:
                            pend_tp.append(r)
                # pending V transposes (1 slot per step keeps PE dense)
                if pend_tp and s % 2 == 1:
                    emit_v_transposes(pend_tp.pop(0))
                # --- cproj from the FIFO, per-group quota ---
                for _ in range(CPROJ_QUOTA[g]):
                    if cproj_fifo:
                        emit_cproj(*cproj_fifo.pop(0))
                # --- softmax tail of group g-1 (PE bcast + DVE norm); late
                # in the step so the bcast never waits on the DVE chain ---
                if g >= 1:
                    emit_group_tail_finish(g - 1, s)
                # tiles of group g-1 become computable once its norm is done
                if s == 2 and g >= 1:
                    pc_, pt2_ = divmod(g - 1, 2)
                    cproj_fifo.extend((pc_, oc, pt2_) for oc in range(8))
                # bunched OT tail at the end of the group
                if s == 7:
                    emit_ot_pair(g, 4)
                    emit_ot_pair(g, 5)
                    emit_ot_pair(g, 6)
                    emit_ot_pair(g, 7)
                    emit_group_tail_recips(g)
                    if g >= 1:
                        for kb in range(8):
                            pts.pop((g - 1, kb), None)

            # end of group: copied cproj tiles -> HBM
            emit_out_dmas()
            while pend_tp:
                emit_v_transposes(pend_tp.pop(0))

        # ---------- epilogue ----------
        # softmax tail of the last group (recips were emitted at g7 s7)
        for h in (0, 1):
            bc = emit_bcast(m1s.pop((7, h)))
            emit_norm(7, h, ots[(7, h)], bc)
        ots.pop((7, 0))
        ots.pop((7, 1))
        # remaining cproj: drain the FIFO, rotating psum tiles through the
        # now-idle S^T/QKV pools so the mm->copy chain pipelines.
        cproj_fifo.extend((3, oc, 1) for oc in range(8))
        for i, (cb3, oc, t2) in enumerate(cproj_fifo):
            pool, tg = ((stP, "st"), (qkvP, "fill"), (msP, "msc"))[i % 3]
            ep = pool.tile([128, 512], F32, tag=tg, name="ep")
            nc.tensor.matmul(
                ep[:],
                wp_sb[:, oc * 128 : (oc + 1) * 128],
                atall[:, cb3 * 1024 + t2 * 512 : cb3 * 1024 + (t2 + 1) * 512],
                start=True,
                stop=True,
            )
            ob = obp.tile([128, 512], F16, tag="ob", name="ob")
            cproj_eng[0] ^= 1
            if cproj_eng[0]:
                nc.vector.tensor_copy(ob[:], ep[:])
            else:
                nc.scalar.copy(ob[:], ep[:])
            cproj_done.append((cb3, oc, t2, ob))
        cproj_fifo.clear()
        emit_out_dmas()


def _build_nc():
    nc = bacc.Bacc(
        "TRN2",
        target_bir_lowering=False,
        debug=False,
        enable_asserts=False,
        num_devices=NCORES,
    )
    xtr = nc.dram_tensor("xtr", [128, 8, 8, 512], BF16, kind="ExternalInput").ap()
    wq = nc.dram_tensor("wq", [128, 1024], BF16, kind="ExternalInput").ap()
    wk = nc.dram_tensor("wk", [128, 1024], BF16, kind="ExternalInput").ap()
    wv = nc.dram_tensor("wv", [128, 1024], BF16, kind="ExternalInput").ap()
    wp = nc.dram_tensor("wp", [128, 1024], BF16, kind="ExternalInput").ap()
    bq = nc.dram_tensor("bq", [128, 1], F32, kind="ExternalInput").ap()
    bk = nc.dram_tensor("bk", [128, 1], F32, kind="ExternalInput").ap()
    bv = nc.dram_tensor("bv", [128, 1], F32, kind="ExternalInput").ap()
    identd = nc.dram_tensor("ident", [128, 128], BF16, kind="ExternalInput").ap()
    onecd = nc.dram_tensor("onec", [128, 64], BF16, kind="ExternalInput").ap()
    onefd = nc.dram_tensor("onef", [128, 64], BF16, kind="ExternalInput").ap()
    out = nc.dram_tensor("out_t", [NX, T], F16, kind="ExternalOutput").ap()
    with tile.TileContext(nc) as tc:
        _emit(nc, tc, xtr, wq, wk, wv, wp, bq, bk, bv, identd, onecd, onefd, out)
    nc.compile()
    return nc


def _pack_w(wcols):
    # [1024, 128] -> [128, 8*128] bf16: sbuf[p, k*128 + j] = W[k*128 + p, j]
    w = np.ascontiguousarray(np.asarray(wcols, dtype=np.float32))
    return np.ascontiguousarray(
        w.reshape(8, 128, 128).transpose(1, 0, 2).reshape(128, 1024)
    ).astype(ml_dtypes.bfloat16)


def _pack_xtr(X):
    # X [T, NX] -> xtr[p, qc, k, n] = X[qc*512+n, k*128+p]
    xt = np.asarray(X, dtype=np.float32).T  # [NX, T]
    xtr = xt.reshape(8, 128, 8, 512).transpose(1, 2, 0, 3)
    return np.ascontiguousarray(xtr).astype(ml_dtypes.bfloat16)


def kernel(hidden_states, c_attn_w, c_attn_b, c_proj_w, c_proj_b):
    global _nc_cache
    hidden_states = np.asarray(hidden_states, dtype=np.float32)
    c_attn_w = np.asarray(c_attn_w, dtype=np.float32)
    c_attn_b = np.asarray(c_attn_b, dtype=np.float32)
    c_proj_w = np.asarray(c_proj_w, dtype=np.float32)
    c_proj_b = np.asarray(c_proj_b, dtype=np.float32)

    if _nc_cache is None:
        _nc_cache = _build_nc()
    nc = _nc_cache

    X = hidden_states.reshape(T, NX)
    xtr_np = _pack_xtr(X)

    in_maps = []
    for c in range(NCORES):
        cs = slice(c * 128, (c + 1) * 128)
        in_maps.append(
            {
                "xtr": xtr_np,
                "wq": _pack_w(c_attn_w[:, c * 128 : (c + 1) * 128]),
                "wk": _pack_w(c_attn_w[:, 1024 + c * 128 : 1024 + (c + 1) * 128]),
                "wv": _pack_w(c_attn_w[:, 2048 + c * 128 : 2048 + (c + 1) * 128]),
                "wp": np.ascontiguousarray(c_proj_w[cs, :]).astype(ml_dtypes.bfloat16),
                "bq": np.ascontiguousarray(c_attn_b[cs].reshape(128, 1)),
                "bk": np.ascontiguousarray(
                    c_attn_b[1024 + c * 128 : 1024 + (c + 1) * 128].reshape(128, 1)
                ),
                "bv": np.ascontiguousarray(
                    c_attn_b[2048 + c * 128 : 2048 + (c + 1) * 128].reshape(128, 1)
                ),
                "ident": np.eye(128, dtype=np.float32).astype(ml_dtypes.bfloat16),
                "onec": np.ones((128, 64), dtype=ml_dtypes.bfloat16),
                "onef": np.full((128, 64), -1.0, dtype=ml_dtypes.bfloat16),
            }
        )

    trace = bool(int(os.environ.get("KERNEL_PROFILE", "0")))
    if trace:
        trace = _ensure_ntff_hook()
    try:
        res = run_bass_kernel_spmd(
            nc, in_maps, core_ids=list(range(NCORES)), trace=trace
        )
    except Exception:
        if not trace:
            raise
        print("kernel.py: traced run failed; retrying untraced")
        res = run_bass_kernel_spmd(nc, in_maps, core_ids=list(range(NCORES)))

    total = np.zeros((NX, T), np.float32)
    for r in res.results:
        total += r["out_t"].astype(np.float32)
    out = total.T.reshape(B, S, NX) + c_proj_b[None, None, :]
    kernel.last_exec_time_ns = res.exec_time_ns
    return out.astype(np.float32)
